# revision 2
# baseline (speedup 1.0000x reference)
"""Trainium2 Bass kernel for CrossAttentionFusion.

Math (the reference's vit_to_graph attention is dead code):
  attn = MHA_graph_to_vit(q_in=graph, kv_in=vit)
  x = concat([vit, attn, anat]) @ Wf.T + bf ; LayerNorm ; ReLU

Host-side folding: the attention output projection Wo is folded into the
fusion block W2 (Wc = W2 @ Wo, bf_total = bf + W2 @ bo), so the kernel
computes per batch element (one NeuronCore each, data-parallel over B=8):
  Q.T = Wq.T-stationary matmuls (graph.T moving)   [d, s] bf16
  K.T likewise from vit.T                          [d, s] bf16
  V   = vit.T-stationary, Wv.T moving (natural)    [s, d] bf16 (+ones col)
  E.T = exp(scale * K_h.T.T @ Q_h.T) per head      [k, q] bf16 (no max sub:
        |logit|max ~3, fp32 exp exact to 1e-5)
  AV  = [V_h|1].T @ E_h -> rows 0-63 = O'.T, row 64 = Z (softmax denom)
  aoT = O'.T / Z (DRAM-bounce partition broadcast of 1/Z)
  x   = aoT.T-stationary @ Wc.T + (vit@W1.T + anat@W3.T + bf_total)
  out = relu((x - mu) * rsqrt(var + eps)) [* g + b if not identity]
"""

import numpy as np
import ml_dtypes

import concourse.bass as bass
import concourse.mybir as mybir
from concourse import bacc
from concourse.tile import TileContext
from concourse.bass_utils import run_bass_kernel_spmd

BF16 = ml_dtypes.bfloat16
F32 = mybir.dt.float32
BF = mybir.dt.bfloat16
AF = mybir.ActivationFunctionType
ALU = mybir.AluOpType

B, S, D, H, HD = 8, 1024, 1024, 16, 64
NT = 8          # 128-row tiles per 1024 dim
NJ = 2          # 512-wide free chunks per 1024
LN_EPS = 1e-5
SCALE = HD ** -0.5

_BUILT = {}


def _build(identity_affine):
    nc = bacc.Bacc("TRN2", target_bir_lowering=False, debug=False, num_devices=B)

    def din(name, shape, dt=BF):
        return nc.dram_tensor(name, shape, dt, kind="ExternalInput")

    vitT = din("vitT", [D, S])
    graphT = din("graphT", [D, S])
    anatT = din("anatT", [D, S])
    wqT = din("wqT", [D, D])
    wkT = din("wkT", [D, D])
    wvT = din("wvT", [D, D])
    wf1T = din("wf1T", [D, D])
    wf3T = din("wf3T", [D, D])
    wcT = din("wcT", [D, D])
    bq_pm = din("bq_pm", [128, NT], F32)
    bk_pm = din("bk_pm", [128, NT], F32)
    bv_bc = din("bv_bc", [128, D], F32)
    bf_bc = din("bf_bc", [128, D], F32)
    if not identity_affine:
        g_bc = din("g_bc", [128, D], F32)
        b_bc = din("b_bc", [128, D], F32)
    out = nc.dram_tensor("out", [S, D], F32, kind="ExternalOutput")

    def tiled(ap):
        # [1024, N] dram -> [128 part, 8, N]
        return ap.ap().rearrange("(t p) n -> p t n", p=128)

    js = [slice(j * 512, (j + 1) * 512) for j in range(NJ)]

    with TileContext(nc) as tc:
        with (
            tc.tile_pool(name="persist", bufs=1) as pp,
            tc.tile_pool(name="zs", bufs=4) as zpool,
            tc.tile_pool(name="stats", bufs=4) as spool,
            tc.tile_pool(name="dram", bufs=1, space="DRAM") as dpool,
            tc.tile_pool(name="psA", bufs=2, space="PSUM") as psA,
            tc.tile_pool(name="psB", bufs=4, space="PSUM") as psB,
        ):
            # ---------- persistent tiles ----------
            vT = pp.tile([128, NT, S], BF, tag="vT")
            nc.sync.dma_start(out=vT, in_=tiled(vitT))
            qT = pp.tile([128, NT, S], BF, tag="qT")
            kT = pp.tile([128, NT, S], BF, tag="kT")
            v_ = pp.tile([128, NT, H, HD + 1], BF, tag="v_")
            nc.vector.memset(v_[:, :, :, HD:HD + 1], 1.0)
            aoT = pp.tile([128, NT, S], BF, tag="aoT")
            xp = pp.tile([128, NT, D], BF, tag="xp")
            bq_s = pp.tile([128, NT], F32, tag="bq")
            nc.sync.dma_start(out=bq_s, in_=bq_pm.ap())
            bk_s = pp.tile([128, NT], F32, tag="bk")
            nc.sync.dma_start(out=bk_s, in_=bk_pm.ap())
            bv_s = pp.tile([128, D], F32, tag="bv")
            nc.sync.dma_start(out=bv_s, in_=bv_bc.ap())
            bf_s = pp.tile([128, D], F32, tag="bf")
            nc.sync.dma_start(out=bf_s, in_=bf_bc.ap())
            if not identity_affine:
                g_s = pp.tile([128, D], F32, tag="g")
                nc.sync.dma_start(out=g_s, in_=g_bc.ap())
                b_s = pp.tile([128, D], F32, tag="b")
                nc.sync.dma_start(out=b_s, in_=b_bc.ap())
            eps_s = pp.tile([128, 1], F32, tag="eps")
            nc.vector.memset(eps_s, LN_EPS)
            z_dram = dpool.tile([H, S], F32, tag="zd")

            # ---------- phase A: QKV projections ----------
            with tc.tile_pool(name="phA", bufs=1) as pa:
                gT = pa.tile([128, NT, S], BF, tag="gT")
                nc.sync.dma_start(out=gT, in_=tiled(graphT))
                wq_s = pa.tile([128, NT, D], BF, tag="wq")
                nc.sync.dma_start(out=wq_s, in_=tiled(wqT))
                wk_s = pa.tile([128, NT, D], BF, tag="wk")
                nc.sync.dma_start(out=wk_s, in_=tiled(wkT))
                wv_s = pa.tile([128, NT, D], BF, tag="wv")
                nc.sync.dma_start(out=wv_s, in_=tiled(wvT))

                # Q.T / K.T: out[d_out, s] ; lhsT = W.T tile, rhs = X.T
                for w_s, x_s, b_s2, dst in ((wq_s, gT, bq_s, qT), (wk_s, vT, bk_s, kT)):
                    for m in range(NT):
                        ps = psA.tile([128, 1024], F32, tag="pa")
                        for ki in range(NT):
                            for j in range(NJ):
                                nc.tensor.matmul(
                                    ps[:, js[j]],
                                    w_s[:, ki, m * 128:(m + 1) * 128],
                                    x_s[:, ki, js[j]],
                                    start=(ki == 0), stop=(ki == NT - 1),
                                )
                        nc.vector.tensor_scalar(
                            out=dst[:, m, :], in0=ps, scalar1=b_s2[:, m:m + 1],
                            scalar2=None, op0=ALU.add,
                        )
                # V natural: lhsT = vit.T tile, rhs = Wv.T
                for m in range(NT):
                    ps = psA.tile([128, 1024], F32, tag="pa")
                    for ki in range(NT):
                        for j in range(NJ):
                            nc.tensor.matmul(
                                ps[:, js[j]],
                                vT[:, ki, m * 128:(m + 1) * 128],
                                wv_s[:, ki, js[j]],
                                start=(ki == 0), stop=(ki == NT - 1),
                            )
                    nc.vector.tensor_tensor(
                        out=v_[:, m, :, 0:HD],
                        in0=ps.rearrange("p (h c) -> p h c", h=H),
                        in1=bv_s.rearrange("p (h c) -> p h c", h=H),
                        op=ALU.add,
                    )

            # ---------- phase B: attention + x_partial ----------
            with (
                tc.tile_pool(name="phB", bufs=1) as pb,
                tc.tile_pool(name="epool", bufs=6) as ep,
            ):
                wf1_s = pb.tile([128, NT, D], BF, tag="wf1")
                nc.sync.dma_start(out=wf1_s, in_=tiled(wf1T))
                wf3_s = pb.tile([128, NT, D], BF, tag="wf3")
                nc.sync.dma_start(out=wf3_s, in_=tiled(wf3T))
                aT = pb.tile([128, NT, S], BF, tag="aT")
                nc.sync.dma_start(out=aT, in_=tiled(anatT))

                for t in range(NT):  # head pair (2t, 2t+1)
                    avp = {}
                    for kt in range(NT):
                        kts = slice(kt * 128, (kt + 1) * 128)
                        pA = psA.tile([128, 1024], F32, tag="pa")
                        pB = psA.tile([128, 1024], F32, tag="pa")
                        for j in range(NJ):
                            nc.tensor.matmul(
                                pA[:, js[j]], kT[0:64, t, kts], qT[0:64, t, js[j]],
                                start=True, stop=True, tile_position=(0, 0),
                            )
                            nc.tensor.matmul(
                                pB[:, js[j]], kT[64:128, t, kts], qT[64:128, t, js[j]],
                                start=True, stop=True, tile_position=(64, 0),
                            )
                        for hl, psc in ((0, pA), (1, pB)):
                            et = ep.tile([128, 1024], BF, tag="e")
                            nc.scalar.activation(out=et, in_=psc, func=AF.Exp,
                                                 scale=SCALE)
                            h = 2 * t + hl
                            for j in range(NJ):
                                if kt == 0:
                                    av_tile = psB.tile([HD + 1, 512], F32, tag="av")
                                    avp[(hl, j)] = av_tile
                                nc.tensor.matmul(
                                    avp[(hl, j)], v_[:, kt, h, :], et[:, js[j]],
                                    start=(kt == 0), stop=(kt == NT - 1),
                                )
                    # evacuate AV: unnormalized O'.T rows + Z row
                    for hl in range(2):
                        h = 2 * t + hl
                        for j in range(NJ):
                            pav = avp[(hl, j)]
                            nc.vector.tensor_copy(
                                aoT[hl * 64:(hl + 1) * 64, t, js[j]], pav[0:HD, :])
                            zt = zpool.tile([1, 512], F32, tag="z")
                            nc.vector.tensor_copy(zt, pav[HD:HD + 1, :])
                            nc.sync.dma_start(out=z_dram[h:h + 1, js[j]], in_=zt)
                    # x_partial for s-tile m = t: vit@W1.T + anat@W3.T + bf
                    ps = psA.tile([128, 1024], F32, tag="pa")
                    for ki in range(NT):
                        for j in range(NJ):
                            nc.tensor.matmul(
                                ps[:, js[j]], vT[:, ki, t * 128:(t + 1) * 128],
                                wf1_s[:, ki, js[j]],
                                start=(ki == 0), stop=False,
                            )
                    for ki in range(NT):
                        for j in range(NJ):
                            nc.tensor.matmul(
                                ps[:, js[j]], aT[:, ki, t * 128:(t + 1) * 128],
                                wf3_s[:, ki, js[j]],
                                start=False, stop=(ki == NT - 1),
                            )
                    nc.vector.tensor_tensor(out=xp[:, t, :], in0=ps, in1=bf_s,
                                            op=ALU.add)

            # ---------- phase C: normalize aoT, fusion pass 2, LN+ReLU ----------
            with (
                tc.tile_pool(name="phC", bufs=1) as pc,
                tc.tile_pool(name="rzp", bufs=2) as rzp,
                tc.tile_pool(name="xout", bufs=3) as xop,
            ):
                wc_s = pc.tile([128, NT, D], BF, tag="wc")
                nc.sync.dma_start(out=wc_s, in_=tiled(wcT))
                for t in range(NT):
                    rzb = rzp.tile([128, 1024], F32, tag="rzb")
                    for hl in range(2):
                        src = bass.AP(
                            tensor=z_dram.tensor,
                            offset=z_dram.offset + (2 * t + hl) * S,
                            ap=[[0, 64], [1, S]],
                        )
                        nc.sync.dma_start(out=rzb[hl * 64:(hl + 1) * 64, :], in_=src)
                    nc.vector.reciprocal(rzb, rzb)
                    nc.vector.tensor_tensor(out=aoT[:, t, :], in0=aoT[:, t, :],
                                            in1=rzb, op=ALU.mult)
                for m in range(NT):
                    ps = psA.tile([128, 1024], F32, tag="pa")
                    for ki in range(NT):
                        for j in range(NJ):
                            nc.tensor.matmul(
                                ps[:, js[j]], aoT[:, ki, m * 128:(m + 1) * 128],
                                wc_s[:, ki, js[j]],
                                start=(ki == 0), stop=(ki == NT - 1),
                            )
                    xt = xop.tile([128, 1024], F32, tag="x")
                    nc.vector.tensor_tensor(out=xt, in0=ps, in1=xp[:, m, :],
                                            op=ALU.add)
                    st = spool.tile([128, 2, 6], F32, tag="st")
                    xr = xt.rearrange("p (a b) -> p a b", a=2)
                    for sg in range(2):
                        nc.vector.bn_stats(out=st[:, sg, :], in_=xr[:, sg, :])
                    mv = spool.tile([128, 2], F32, tag="mv")
                    nc.vector.bn_aggr(out=mv, in_=st)
                    rstd = spool.tile([128, 1], F32, tag="rstd")
                    nc.scalar.activation(out=rstd, in_=mv[:, 1:2], func=AF.Sqrt,
                                         bias=eps_s, scale=1.0)
                    nc.vector.reciprocal(rstd, rstd)
                    if identity_affine:
                        nmr = spool.tile([128, 1], F32, tag="nmr")
                        nc.vector.tensor_scalar(
                            out=nmr, in0=mv[:, 0:1], scalar1=rstd, scalar2=-1.0,
                            op0=ALU.mult, op1=ALU.mult,
                        )
                        nc.scalar.activation(out=xt, in_=xt, func=AF.Relu,
                                             bias=nmr, scale=rstd)
                    else:
                        nc.vector.tensor_scalar(
                            out=xt, in0=xt, scalar1=mv[:, 0:1], scalar2=rstd,
                            op0=ALU.subtract, op1=ALU.mult,
                        )
                        nc.vector.tensor_tensor(out=xt, in0=xt, in1=g_s,
                                                op=ALU.mult)
                        nc.vector.tensor_tensor(out=xt, in0=xt, in1=b_s,
                                                op=ALU.add)
                        nc.vector.tensor_scalar(
                            out=xt, in0=xt, scalar1=0.0, scalar2=None, op0=ALU.max)
                    nc.sync.dma_start(out=out.ap()[m * 128:(m + 1) * 128, :], in_=xt)

    nc.compile()
    return nc


def kernel(**inputs):
    vit = np.asarray(inputs["vit_features"], dtype=np.float32)
    graph = np.asarray(inputs["graph_features"], dtype=np.float32)
    anat = np.asarray(inputs["anatomical_features"], dtype=np.float32)
    p = inputs["params"]
    g2v = p["graph_to_vit"]
    fus = p["fusion"]
    wq = np.asarray(g2v["q"]["w"], np.float32)
    bq = np.asarray(g2v["q"]["b"], np.float32)
    wk = np.asarray(g2v["k"]["w"], np.float32)
    bk = np.asarray(g2v["k"]["b"], np.float32)
    wv = np.asarray(g2v["v"]["w"], np.float32)
    bv = np.asarray(g2v["v"]["b"], np.float32)
    wo = np.asarray(g2v["o"]["w"], np.float32)
    bo = np.asarray(g2v["o"]["b"], np.float32)
    wf = np.asarray(fus["lin"]["w"], np.float32)
    bf = np.asarray(fus["lin"]["b"], np.float32)
    ln_g = np.asarray(fus["ln_g"], np.float32)
    ln_b = np.asarray(fus["ln_b"], np.float32)

    identity_affine = bool(np.all(ln_g == 1.0) and np.all(ln_b == 0.0))
    if identity_affine not in _BUILT:
        _BUILT[identity_affine] = _build(identity_affine)
    nc = _BUILT[identity_affine]

    W1 = wf[:, :D]
    W2 = wf[:, D:2 * D]
    W3 = wf[:, 2 * D:]
    Wc = W2 @ wo
    bf_total = bf + W2 @ bo

    def bfT(w):  # [out, in] weight -> transposed bf16 [in, out]
        return np.ascontiguousarray(w.T).astype(BF16)

    def pm(b):  # [1024] bias -> [128, 8] partition-major f32
        return np.ascontiguousarray(b.reshape(NT, 128).T).astype(np.float32)

    def bc(b):  # [1024] row -> [128, 1024] broadcast f32
        return np.ascontiguousarray(np.broadcast_to(b, (128, D))).astype(np.float32)

    shared = {
        "wqT": bfT(wq), "wkT": bfT(wk), "wvT": bfT(wv),
        "wf1T": bfT(W1), "wf3T": bfT(W3), "wcT": bfT(Wc),
        "bq_pm": pm(bq), "bk_pm": pm(bk),
        "bv_bc": bc(bv), "bf_bc": bc(bf_total),
    }
    if not identity_affine:
        shared["g_bc"] = bc(ln_g)
        shared["b_bc"] = bc(ln_b)

    in_maps = []
    for c in range(B):
        m = dict(shared)
        m["vitT"] = np.ascontiguousarray(vit[c].T).astype(BF16)
        m["graphT"] = np.ascontiguousarray(graph[c].T).astype(BF16)
        m["anatT"] = np.ascontiguousarray(anat[c].T).astype(BF16)
        in_maps.append(m)

    res = run_bass_kernel_spmd(nc, in_maps, core_ids=list(range(B)))
    return np.stack([res.results[c]["out"] for c in range(B)]).astype(np.float32)
